# revision 68
# baseline (speedup 1.0000x reference)
"""Two-layer GCN (PyG GCNConv x2, eval mode) on 8 Trainium2 NeuronCores.

out = S @ (relu(S @ (x@W1) + b1) @ W2) + b2,  S = D^-1/2 (A+I) D^-1/2

Destination-sharded design (v3):
  - 50000 nodes sharded 6250/core; per core, destinations are permuted into
    49 blocks of 128 balanced by in-degree (host permutation, undone on the
    way out), so the SPMD program's per-block tile counts carry ~0.2%
    padding instead of ~13%.
  - h' = dinv*(x@W1) is computed sharded in bf16 (x arrives host-transposed
    so phase A is matmul+activation only) and AllGathered into a 256B-row
    table in Shared DRAM split into two chunks. The two layers use
    DIFFERENT chunk splits: layer 1's table has a SMALL chunk0 (20 blocks,
    fires ~40% into phase A, so L1 gathers start early) while layer 2's has
    a SMALL chunk1 (17 blocks, minimizing the serial AllGather tail after
    layer 1). A tiny warmup AllReduce at program start absorbs cross-core
    arrival skew before the first real collective.
  - Aggregation per 128-dest block: dma_gather source rows (<=1024 idx per
    call — the Q7 ucode dies above that, verified — round-robined over 4
    SWDGE queues), one-hot selectors built 1 op/block via a broadcast-AP
    is_equal (all-bf16), then matmul(lhsT=onehot[e,d], rhs=msgs[e,f])
    accumulating in PSUM. Gather calls for the first table chunk are
    emitted LAG groups ahead of second-chunk calls so a wait on the second
    AllGather can't head-of-line-block the in-order gpsimd queue.
    The reference's added self-loops never touch the gather path: an
    identity-selector matmul adds the block's own rows from a persistent
    SBUF copy (h1own/h2own), so they are never re-read from DRAM.
  - Layer 1 tail: h2pre = dinv*relu(dinv*G) = relu(dinv^2*G) in one
    ScalarE op (b1==0); AllGather -> layer-2 table of h2pre rows; layer 2
    aggregates h2pre and applies W2 after aggregation:
    out = dinv*(G2)@W2 + b2, accumulated feature-major in SBUF, stored in
    two bulk DMAs, and transposed on host.
  - int16 gather indices force a lo/hi base split between the two chunks.
"""

import math
from contextlib import ExitStack

import numpy as np

NC = 8
P = 128
GROUP = 4  # dest blocks per gather buffer
MAXCALL = 8  # tiles per dma_gather call (1024 idx ucode limit)
NQUEUES = 4
PAD_DEST = 200  # destid for padding edges; never matches iota 0..127
DEBUG_STAGE = "full"  # "A" | "AG1" | "L1" | "AG2" | "full"


def _pack_idx(v: np.ndarray) -> np.ndarray:
    """[T*128] int -> [128, 8T] int16 in dma_gather's wrap-16 layout,
    replicated over the 8 gpsimd cores (element i lives at [i%16, i//16])."""
    assert v.size % P == 0
    a = v.reshape(-1, 16).T.astype(np.int16)  # [16, 8T]
    return np.tile(a, (8, 1))  # [128, 8T]


def _balance_blocks(weights: np.ndarray, nblk: int) -> np.ndarray:
    """Assign len(weights) items into nblk blocks of <=128, balancing block
    weight sums. Returns pos[i] = block*128 + slot."""
    import heapq

    n = weights.size
    order = np.argsort(-weights, kind="stable")
    loads = np.zeros(nblk, dtype=np.int64)
    fill = np.zeros(nblk, dtype=np.int64)
    cap = np.full(nblk, P, dtype=np.int64)
    cap[nblk - 1] = n - (nblk - 1) * P  # last block holds the remainder
    pos = np.empty(n, dtype=np.int64)
    heap = [(0, b) for b in range(nblk)]
    heapq.heapify(heap)
    for i in order:
        while True:
            load, b = heapq.heappop(heap)
            if fill[b] < cap[b]:
                break
        pos[i] = b * P + fill[b]
        fill[b] += 1
        loads[b] = load + weights[i]
        if fill[b] < cap[b]:
            heapq.heappush(heap, (int(loads[b]), b))
    return pos


def _preprocess(x, edge_index, W1, b1, W2, b2):
    import ml_dtypes

    N, Fin = x.shape
    Fh = W1.shape[1]
    Fout = W2.shape[1]
    assert N % NC == 0
    NPC = N // NC
    NBLK = math.ceil(NPC / P)

    row = np.asarray(edge_index[0], dtype=np.int64)
    col = np.asarray(edge_index[1], dtype=np.int64)

    # degrees include the self-loop the reference appends to every node
    deg = (np.bincount(col, minlength=N) + 1).astype(np.float64)
    dinv = (1.0 / np.sqrt(deg)).astype(np.float32)

    core = col // NPC

    # per-core balanced permutation of destination slots by in-edge count
    cnt_in = np.bincount(col, minlength=N)
    pos_in_core = np.empty(N, dtype=np.int64)
    node_at = np.empty((NC, NPC), dtype=np.int64)
    for c in range(NC):
        w = cnt_in[c * NPC : (c + 1) * NPC]
        p = _balance_blocks(w, NBLK)
        pos_in_core[c * NPC : (c + 1) * NPC] = p
        node_at[c][p] = np.arange(NPC) + c * NPC

    # chunk-interleaved global row of every node in the gather tables:
    # AllGather chunk k covers core-local rows [r0,r1); its output holds the
    # 8 cores' slices consecutively at table rows [8*r0, 8*r1).
    # The two layers use DIFFERENT chunk splits: layer 1 wants a small chunk0
    # (it gates the L1 gather start right after phase A), layer 2 wants a
    # small chunk1 (it is the serial AllGather tail after layer 1).
    groups0 = [list(range(g, min(g + GROUP, NBLK))) for g in range(0, NBLK, GROUP)]
    blk = pos_in_core[col] >> 7
    dloc = pos_in_core[col] & 127

    def build_layout(cg):
        """Bucketing for one layer's table split [cg, rest] (in groups)."""
        chunks = []
        chunk_last_group = []
        for g0, g1 in ((0, cg), (cg, len(groups0))):
            blocks = [b for gr in groups0[g0:g1] for b in gr]
            r0 = blocks[0] * P
            r1 = min(blocks[-1] * P + P, NPC)
            chunks.append((r0, r1 - r0))
            chunk_last_group.append(g1 - 1)
        chunk_of_row = np.empty(NPC, dtype=np.int64)
        for k, (r0, ln) in enumerate(chunks):
            chunk_of_row[r0 : r0 + ln] = k
        k_of = chunk_of_row[pos_in_core]
        r0_of = np.array([c[0] for c in chunks])[k_of]
        len_of = np.array([c[1] for c in chunks])[k_of]
        gpos_node = (
            NC * r0_of + (np.arange(N) // NPC) * len_of + (pos_in_core - r0_of)
        )
        for r0k, lnk in chunks:
            assert NC * lnk <= 32768, "table half exceeds int16 index range"

        seg = k_of[row]  # which table half the source row lives in
        src_idx = gpos_node[row] - NC * r0_of[row]
        key = (core * NBLK + blk) * 2 + seg
        order = np.argsort(key, kind="stable")
        skey = key[order]
        nbuck = NC * NBLK * 2
        starts = np.searchsorted(skey, np.arange(nbuck))
        ends = np.searchsorted(skey, np.arange(nbuck) + 1)
        cnt = (ends - starts).reshape(NC, NBLK, 2)
        T_LO = np.maximum(1, np.ceil(cnt[:, :, 0] / P).max(axis=0).astype(np.int64))
        T_HI = np.maximum(1, np.ceil(cnt[:, :, 1] / P).max(axis=0).astype(np.int64))
        return dict(
            chunks=chunks,
            chunk_last_group=chunk_last_group,
            ssrc=src_idx[order],
            sdloc=dloc[order],
            starts=starts,
            ends=ends,
            T_LO=T_LO,
            T_HI=T_HI,
            T_consume=int((T_LO + T_HI).sum()),
        )

    ng = len(groups0)
    cg2 = min(math.ceil(ng / 2) + 1, ng - 1)
    while cg2 * GROUP * P * NC > 32768:
        cg2 -= 1
    cg1 = ng - cg2  # small chunk0 for layer 1 (mirror of layer 2's split)
    while (ng - cg1) * GROUP * P * NC > 32768:
        cg1 += 1
    lays = [build_layout(cg1), build_layout(cg2)]

    groups = groups0

    in_maps = []
    w1bf = np.asarray(W1, dtype=ml_dtypes.bfloat16)
    w2bf = np.asarray(W2, dtype=ml_dtypes.bfloat16)
    b1f = np.asarray(b1, dtype=np.float32)
    b2f = np.asarray(b2, dtype=np.float32)
    b1b = np.broadcast_to(b1f[None, :], (P, Fh)).copy()
    b2c = np.ascontiguousarray(b2f[:, None])  # [Fout, 1]

    for c in range(NC):
        did_parts = []
        idx_cols = []
        for lay in lays:
            TSEG = (lay["T_LO"], lay["T_HI"])
            starts, ends = lay["starts"], lay["ends"]
            ssrc, sdloc = lay["ssrc"], lay["sdloc"]
            did = np.full((P, lay["T_consume"]), PAD_DEST, dtype=np.float32)
            ccol = 0
            for b in range(NBLK):
                for sg in (0, 1):
                    bidx = (c * NBLK + b) * 2 + sg
                    n = ends[bidx] - starts[bidx]
                    T = int(TSEG[sg][b])
                    if T == 0:
                        assert n == 0
                        continue
                    tmp = np.full(T * P, PAD_DEST, dtype=np.float32)
                    tmp[:n] = sdloc[starts[bidx] : ends[bidx]]
                    did[:, ccol : ccol + T] = tmp.reshape(T, P).T
                    ccol += T
            assert ccol == lay["T_consume"]
            did_parts.append(did)

            for blocks in groups:
                for sg in (0, 1):
                    for b in blocks:
                        T = int(TSEG[sg][b])
                        if T == 0:
                            continue
                        bidx = (c * NBLK + b) * 2 + sg
                        n = ends[bidx] - starts[bidx]
                        s = ssrc[starts[bidx] : ends[bidx]]
                        tmp = np.zeros(T * P, dtype=np.int64)
                        tmp[:n] = s
                        idx_cols.append(_pack_idx(tmp))
        did = np.concatenate(did_parts, axis=1)
        idx = np.concatenate(idx_cols, axis=1)

        # dinv and dinv^2 columns at permuted positions (pad 1.0)
        dvflat = np.ones(NBLK * P, dtype=np.float32)
        dvflat[pos_in_core[c * NPC : (c + 1) * NPC]] = dinv[c * NPC : (c + 1) * NPC]
        dvc = np.ascontiguousarray(dvflat.reshape(NBLK, P).T)
        dv2c = np.ascontiguousarray(dvc * dvc)

        xs = np.zeros((NBLK * P, Fin), dtype=ml_dtypes.bfloat16)
        xs[:NPC] = x[node_at[c]].astype(ml_dtypes.bfloat16)
        # host-transposed: [Fin, NBLK*P]; block b's lhsT = [:, b*P:(b+1)*P],
        # so phase A needs no PE transpose / PSUM round-trip
        xsw = np.ascontiguousarray(xs.T)

        in_maps.append(
            {
                "xs": xsw,
                "w1": w1bf,
                "w2": w2bf,
                "b1b": b1b,
                "b2c": b2c,
                "dv": dvc,
                "dv2": dv2c,
                "idx": np.ascontiguousarray(idx),
                "did": did.astype(ml_dtypes.bfloat16),
            }
        )

    meta = dict(
        N=N,
        Fin=Fin,
        Fh=Fh,
        Fout=Fout,
        NPC=NPC,
        NBLK=NBLK,
        T_LO=[[int(t) for t in lay["T_LO"]] for lay in lays],
        T_HI=[[int(t) for t in lay["T_HI"]] for lay in lays],
        T_MAX=int(
            max(
                int(lay["T_LO"][b]) + int(lay["T_HI"][b])
                for lay in lays
                for b in range(NBLK)
            )
        ),
        groups=groups,
        chunks=[lay["chunks"] for lay in lays],
        chunk_last_group=[lay["chunk_last_group"] for lay in lays],
        idx_w=in_maps[0]["idx"].shape[1],
        T_consume=[lay["T_consume"] for lay in lays],
        has_b1=bool(np.any(b1f)),
    )
    return in_maps, meta, node_at


def _build_program(meta):
    import concourse.bacc as bacc
    import concourse.mybir as mybir
    import concourse.tile as tile
    from concourse.masks import make_identity

    N = meta["N"]
    Fin, Fh, Fout = meta["Fin"], meta["Fh"], meta["Fout"]
    NPC, NBLK = meta["NPC"], meta["NBLK"]
    groups = meta["groups"]
    TW = Fh  # table row width: 256B bf16 rows

    f32 = mybir.dt.float32
    bf16 = mybir.dt.bfloat16
    i16 = mybir.dt.int16
    i32 = mybir.dt.int32

    nc = bacc.Bacc(
        "TRN2", target_bir_lowering=False, debug=True, num_swdge_queues=NQUEUES
    )

    xs = nc.dram_tensor("xs", [Fin, NBLK * P], bf16, kind="ExternalInput")
    w1 = nc.dram_tensor("w1", [Fin, Fh], bf16, kind="ExternalInput")
    w2 = nc.dram_tensor("w2", [Fh, Fout], bf16, kind="ExternalInput")
    b1b = nc.dram_tensor("b1b", [P, Fh], f32, kind="ExternalInput")
    b2c = nc.dram_tensor("b2c", [Fout, 1], f32, kind="ExternalInput")
    dv = nc.dram_tensor("dv", [P, NBLK], f32, kind="ExternalInput")
    dv2 = nc.dram_tensor("dv2", [P, NBLK], f32, kind="ExternalInput")
    idxd = nc.dram_tensor("idx", [P, meta["idx_w"]], i16, kind="ExternalInput")
    did_w = sum(meta["T_consume"])
    didd = nc.dram_tensor("did", [P, did_w], bf16, kind="ExternalInput")
    out = nc.dram_tensor("out", [Fout, NPC], f32, kind="ExternalOutput")

    qctr = [0]

    with tile.TileContext(nc, num_cores=NC) as tc, ExitStack() as ctx:
        consts = ctx.enter_context(tc.tile_pool(name="consts", bufs=1))
        dram = ctx.enter_context(tc.tile_pool(name="dram", bufs=1, space="DRAM"))
        wpool = ctx.enter_context(tc.tile_pool(name="work", bufs=4))
        hpool = ctx.enter_context(tc.tile_pool(name="hp", bufs=4))
        ownp = ctx.enter_context(tc.tile_pool(name="hown", bufs=1))
        selp = ctx.enter_context(tc.tile_pool(name="sel", bufs=3))
        glo = ctx.enter_context(tc.tile_pool(name="glo", bufs=5))
        ghi = ctx.enter_context(tc.tile_pool(name="ghi", bufs=2))
        part = ctx.enter_context(tc.tile_pool(name="part", bufs=24))
        pg = ctx.enter_context(tc.tile_pool(name="pg", bufs=3, space="PSUM"))
        pt = ctx.enter_context(tc.tile_pool(name="pt", bufs=3, space="PSUM"))
        ph = ctx.enter_context(tc.tile_pool(name="ph", bufs=2, space="PSUM"))

        # ---- constants
        ident = consts.tile([P, P], f32)
        make_identity(nc, ident[:])
        ident_bf = consts.tile([P, P], bf16)
        nc.vector.tensor_copy(ident_bf[:], ident[:])
        iota_i = consts.tile([P, meta["T_MAX"], P], i32)
        nc.gpsimd.iota(
            iota_i[:], pattern=[[0, meta["T_MAX"]], [1, P]], base=0,
            channel_multiplier=0,
        )
        iota_f = consts.tile([P, meta["T_MAX"], P], bf16)
        nc.vector.tensor_copy(iota_f[:], iota_i[:])
        w1t = consts.tile([Fin, Fh], bf16)
        nc.sync.dma_start(w1t[:], w1[:])
        w2t = consts.tile([Fh, Fout], bf16)
        nc.sync.dma_start(w2t[:], w2[:])
        dvt = consts.tile([P, NBLK], f32)
        nc.sync.dma_start(dvt[:], dv[:])
        dv2t = consts.tile([P, NBLK], f32)
        nc.sync.dma_start(dv2t[:], dv2[:])
        didt = consts.tile([P, did_w], bf16)
        nc.sync.dma_start(didt[:], didd[:])
        idxt = consts.tile([P, meta["idx_w"]], i16)
        nc.sync.dma_start(idxt[:], idxd[:])
        b1t = consts.tile([P, Fh], f32)
        nc.sync.dma_start(b1t[:], b1b[:])
        b2t = consts.tile([Fout, 1], f32)
        nc.sync.dma_start(b2t[:], b2c[:])
        xst = consts.tile([Fin, NBLK * P], bf16)
        nc.sync.dma_start(xst[:], xs[:])
        outsb = consts.tile([Fout, NPC], f32)

        h1s = dram.tile([NPC, TW], bf16)
        h2s = dram.tile([NPC, TW], bf16)
        # warmup collective: absorbs cross-core arrival skew + ncfw startup
        # while constants load, so AG1 chunk 0 starts without a sync delay
        wu_in = dram.tile([1, P], bf16, name="wu_in")
        wu_out = dram.tile([1, P], bf16, addr_space="Shared", name="wu_out")
        nc.gpsimd.collective_compute(
            "AllReduce",
            mybir.AluOpType.add,
            replica_groups=[list(range(NC))],
            ins=[wu_in[:]],
            outs=[wu_out[:]],
        )
        # persistent SBUF copies of this core's own h rows (self-loop rhs);
        # avoids per-block 256B-descriptor DMA reloads from h1s/h2s
        h1own = ownp.tile([P, NBLK * Fh], bf16)
        h2own = ownp.tile([P, NBLK * Fh], bf16)
        t1h = [
            dram.tile([NC * ln, TW], bf16, addr_space="Shared", name=f"t1h{k}")
            for k, (_, ln) in enumerate(meta["chunks"][0])
        ]
        t2h = [
            dram.tile([NC * ln, TW], bf16, addr_space="Shared", name=f"t2h{k}")
            for k, (_, ln) in enumerate(meta["chunks"][1])
        ]

        def ag_chunk(shard, halves, chunks, k):
            r0, ln = chunks[k]
            nc.gpsimd.collective_compute(
                "AllGather",
                mybir.AluOpType.bypass,
                replica_groups=[list(range(NC))],
                ins=[shard[r0 : r0 + ln, :]],
                outs=[halves[k].opt()],
            )

        # ---- phase A: h' = dinv * (x @ W1), bf16, 256B rows;
        # AllGather chunk k fires as soon as its group's blocks are stored.
        for k, blocks in enumerate(groups):
            for b in blocks:
                nv = min(P, NPC - b * P)
                hps = pg.tile([P, Fh], f32, tag="pg")
                nc.tensor.matmul(
                    hps[:], lhsT=xst[:, b * P : (b + 1) * P], rhs=w1t[:],
                    start=True, stop=True,
                )
                hp = hpool.tile([P, TW], bf16, tag="hp")
                # alternate engine assignment per block so scalar and vector
                # each carry one of the two PSUM drains — halves the
                # per-block latency that gates the first AllGather chunk
                if b % 2 == 0:
                    nc.scalar.activation(
                        hp[:, 0:Fh], hps[:], mybir.ActivationFunctionType.Copy,
                        scale=dvt[:, b : b + 1],
                    )
                    nc.vector.tensor_scalar(
                        h1own[:, b * Fh : (b + 1) * Fh], hps[:],
                        dvt[:, b : b + 1], None, mybir.AluOpType.mult,
                    )
                else:
                    nc.vector.tensor_scalar(
                        hp[:, 0:Fh], hps[:],
                        dvt[:, b : b + 1], None, mybir.AluOpType.mult,
                    )
                    nc.scalar.activation(
                        h1own[:, b * Fh : (b + 1) * Fh], hps[:],
                        mybir.ActivationFunctionType.Copy,
                        scale=dvt[:, b : b + 1],
                    )
                nc.sync.dma_start(h1s[b * P : b * P + nv, :], hp[:nv, :])
            if DEBUG_STAGE in ("AG1", "L1", "AG2", "full"):
                if k in meta["chunk_last_group"][0]:
                    ag_chunk(
                        h1s, t1h, meta["chunks"][0],
                        meta["chunk_last_group"][0].index(k),
                    )

        # per-layer tile/idx/did offsets: layer 2's tables follow layer 1's
        l2_tiles = sum(
            meta["T_LO"][0][b] + meta["T_HI"][0][b] for b in range(NBLK)
        )

        def run_layer(layer):
            li = layer - 1
            T_LO, T_HI = meta["T_LO"][li], meta["T_HI"][li]
            halves = t1h if layer == 1 else t2h
            lo_ap = halves[0][:]
            hi_ap = halves[1][:] if len(halves) > 1 else None
            idx_off = 0 if layer == 1 else l2_tiles  # tiles, gather order

            def chunked_gather(buf, src_ap, ntiles, idx_off):
                c0 = 0
                while c0 < ntiles:
                    cn = min(MAXCALL, ntiles - c0)
                    nc.gpsimd.dma_gather(
                        buf[:, c0 : c0 + cn, :], src_ap,
                        idxt[:, 8 * (idx_off + c0) : 8 * (idx_off + c0 + cn)],
                        P * cn, P * cn, TW,
                        queue_num=qctr[0] % NQUEUES,
                    )
                    qctr[0] += 1
                    c0 += cn

            # software-pipeline gather emission: issue lo gathers LAG groups
            # ahead of hi gathers, so a hi call's wait on the table's second
            # AllGather chunk can't head-of-line-block the runnable lo stream
            # on the in-order gpsimd queue.
            offs = []
            o = idx_off
            for blocks in groups:
                tlo_g = sum(T_LO[b] for b in blocks)
                thi_g = sum(T_HI[b] for b in blocks)
                offs.append((o, o + tlo_g, tlo_g, thi_g))
                o += tlo_g + thi_g
            idx_off = o
            lobs = [None] * len(groups)

            def emit_lo(gk):
                lo_off, _, tlo_g, _ = offs[gk]
                lob = glo.tile([P, tlo_g, TW], bf16, tag="glo")
                chunked_gather(lob, lo_ap, tlo_g, lo_off)
                lobs[gk] = lob

            def emit_group(gk, blocks, split):
                lo_off, hi_off, tlo_g, thi_g = offs[gk]
                lob = lobs[gk]
                hib = None
                if thi_g > 0:
                    hib = ghi.tile([P, thi_g, TW], bf16, tag="ghi")
                    chunked_gather(hib, hi_ap, thi_g, hi_off)
                consume(gk, blocks, lob, hib, split)

            # per-block selector-plane offsets into didt (lo planes first,
            # then hi planes, per block)
            bdcol = []
            o2 = 0 if layer == 1 else meta["T_consume"][0]
            for b in range(NBLK):
                bdcol.append(o2)
                o2 += T_LO[b] + T_HI[b]

            def build_sel(c0, n):
                sel = selp.tile([P, n, P], bf16, tag="sel", name=f"sel_{c0}")
                nc.vector.tensor_tensor(
                    sel[:],
                    didt[:, c0 : c0 + n].to_broadcast([P, n, P]),
                    iota_f[:, 0:n, :],
                    mybir.AluOpType.is_equal,
                )
                return sel

            partials = {}

            def consume_lo(gk, blocks):
                """Split path: aggregate the lo half into a bf16 partial as
                soon as lob lands, releasing the lob buffer early and keeping
                the PE busy while the second AllGather chunk is in flight."""
                lob = lobs[gk]
                lo_t = 0
                for b in blocks:
                    ntl = T_LO[b]
                    sel = build_sel(bdcol[b], ntl)
                    own = (h1own if layer == 1 else h2own)[
                        :, b * Fh : (b + 1) * Fh
                    ]
                    acc = pg.tile([P, Fh], f32, tag="pg")
                    nc.tensor.matmul(
                        acc[:], lhsT=ident_bf[:], rhs=own,
                        start=True, stop=False,
                    )
                    for t in range(ntl):
                        nc.tensor.matmul(
                            acc[:], lhsT=sel[:, t, :],
                            rhs=lob[:, lo_t + t, 0:Fh],
                            start=False, stop=(t == ntl - 1),
                        )
                    pb = part.tile([P, Fh], bf16, tag="part", name=f"pb{b}")
                    nc.vector.tensor_copy(pb[:], acc[:])
                    partials[b] = pb
                    lo_t += ntl

            def consume(gk, blocks, lob, hib, split):
                lo_t = 0
                hi_t = 0
                for b in blocks:
                    nv = min(P, NPC - b * P)
                    ntile = T_LO[b] + T_HI[b]

                    acc = pg.tile([P, Fh], f32, tag="pg")
                    if split:
                        # resume from the parked lo-partial
                        sel = build_sel(bdcol[b] + T_LO[b], T_HI[b])
                        nc.tensor.matmul(
                            acc[:], lhsT=ident_bf[:], rhs=partials[b][:],
                            start=True, stop=False,
                        )
                        k = 0
                        tiles = ((hib, hi_t, T_HI[b]),)
                        klast = T_HI[b] - 1
                    else:
                        sel = build_sel(bdcol[b], ntile)
                        own = (h1own if layer == 1 else h2own)[
                            :, b * Fh : (b + 1) * Fh
                        ]
                        nc.tensor.matmul(
                            acc[:], lhsT=ident_bf[:], rhs=own,
                            start=True, stop=False,
                        )
                        k = 0
                        tiles = ((lob, lo_t, T_LO[b]), (hib, hi_t, T_HI[b]))
                        klast = ntile - 1
                    for buf, t0, tn in tiles:
                        for t in range(tn):
                            nc.tensor.matmul(
                                acc[:],
                                lhsT=sel[:, k, :],
                                rhs=buf[:, t0 + t, 0:Fh],
                                start=False,
                                stop=(k == klast),
                            )
                            k += 1
                    lo_t += T_LO[b]
                    hi_t += T_HI[b]

                    if layer == 1:
                        # h2pre = dinv*relu(dinv*G + b1) = relu(dinv^2*G), b1=0
                        hp = hpool.tile([P, TW], bf16, tag="hp")
                        if meta["has_b1"]:
                            tmp = wpool.tile([P, Fh], f32, tag="l1tmp")
                            nc.vector.tensor_scalar(
                                tmp[:], acc[:], dvt[:, b : b + 1], None,
                                mybir.AluOpType.mult,
                            )
                            nc.vector.tensor_tensor(
                                tmp[:], tmp[:], b1t[:], mybir.AluOpType.add
                            )
                            nc.scalar.activation(
                                hp[:, 0:Fh], tmp[:],
                                mybir.ActivationFunctionType.Relu,
                                scale=dvt[:, b : b + 1],
                            )
                        else:
                            nc.scalar.activation(
                                hp[:, 0:Fh], acc[:],
                                mybir.ActivationFunctionType.Relu,
                                scale=dv2t[:, b : b + 1],
                            )
                        if meta["has_b1"]:
                            nc.vector.tensor_copy(
                                h2own[:, b * Fh : (b + 1) * Fh], hp[:, 0:Fh]
                            )
                        else:
                            nc.scalar.activation(
                                h2own[:, b * Fh : (b + 1) * Fh], acc[:],
                                mybir.ActivationFunctionType.Relu,
                                scale=dv2t[:, b : b + 1],
                            )
                        nc.sync.dma_start(
                            h2s[b * P : b * P + nv, :], hp[:nv, :]
                        )
                    else:
                        # out.T[:, block] = W2.T @ (dinv*G2).T + b2
                        r2 = wpool.tile([P, Fh], f32, tag="l2r")
                        nc.scalar.activation(
                            r2[:], acc[:], mybir.ActivationFunctionType.Copy,
                            scale=dvt[:, b : b + 1],
                        )
                        r2T_ps = pt.tile([P, P], f32, tag="pt")
                        nc.tensor.transpose(r2T_ps[:], r2[:], ident[:])
                        r2T = wpool.tile([P, P], bf16, tag="wbf")
                        nc.vector.tensor_copy(r2T[:], r2T_ps[:])
                        o2T_ps = ph.tile([Fout, P], f32, tag="ph")
                        nc.tensor.matmul(
                            o2T_ps[:], lhsT=w2t[:], rhs=r2T[:],
                            start=True, stop=True,
                        )
                        nc.scalar.activation(
                            outsb[:, b * P : b * P + nv], o2T_ps[:, :nv],
                            mybir.ActivationFunctionType.Identity,
                            bias=b2t[:, 0:1],
                        )
                        if b == 31:
                            nc.sync.dma_start(
                                out[:, : 32 * P], outsb[:, : 32 * P]
                            )
                if layer == 1 and DEBUG_STAGE in ("AG2", "full"):
                    if gk in meta["chunk_last_group"][1]:
                        ag_chunk(
                            h2s, t2h, meta["chunks"][1],
                            meta["chunk_last_group"][1].index(gk),
                        )

            LAG = 3
            SPLIT = 5  # groups using split lo/hi consumption
            ng = len(groups)
            for gk in range(min(LAG, ng)):
                emit_lo(gk)
                if gk < SPLIT:
                    consume_lo(gk, groups[gk])
            for gk, blocks in enumerate(groups):
                if gk + LAG < ng:
                    emit_lo(gk + LAG)
                    if gk + LAG < SPLIT:
                        consume_lo(gk + LAG, groups[gk + LAG])
                emit_group(gk, blocks, gk < SPLIT)

        if DEBUG_STAGE in ("L1", "AG2", "full"):
            run_layer(1)
        if DEBUG_STAGE == "full":
            run_layer(2)
            nc.sync.dma_start(out[:, 32 * P :], outsb[:, 32 * P :])

        if DEBUG_STAGE != "full":
            # debug: write finite bytes into `out` derived from stage product
            src = {"A": h1s, "AG1": t1h[0], "L1": h2s, "AG2": t2h[0]}[DEBUG_STAGE]
            gdb = wpool.tile([P, Fout], bf16, tag="wbf")
            nc.sync.dma_start(gdb[:], src[0:P, 0:Fout])
            red = wpool.tile([P, 1], f32, tag="l1tmp")
            nc.vector.reduce_sum(red[:], gdb[:], axis=mybir.AxisListType.X)
            odb = wpool.tile([Fout, NPC], f32, tag="l2o")
            nc.vector.memset(odb[:], 0.0)
            nc.vector.tensor_copy(odb[0:1, 0:1], red[0:1, :])
            nc.sync.dma_start(out[:], odb[:])

    nc.compile()
    return nc


def _assemble(results, meta, node_at):
    N, Fout = meta["N"], meta["Fout"]
    out = np.empty((N, Fout), dtype=np.float32)
    for c in range(NC):
        out[node_at[c]] = np.asarray(results[c]["out"]).T
    return out


def kernel(**inputs) -> np.ndarray:
    x = np.asarray(inputs["x"])
    edge_index = np.asarray(inputs["edge_index"])
    W1 = np.asarray(inputs["W1"])
    b1 = np.asarray(inputs["b1"])
    W2 = np.asarray(inputs["W2"])
    b2 = np.asarray(inputs["b2"])

    in_maps, meta, node_at = _preprocess(x, edge_index, W1, b1, W2, b2)
    nc = _build_program(meta)

    from concourse.bass_utils import run_bass_kernel_spmd

    res = run_bass_kernel_spmd(nc, in_maps, list(range(NC)))
    return _assemble(res.results, meta, node_at)



# revision 69
# speedup vs baseline: 1.0146x; 1.0146x over previous
"""Two-layer GCN (PyG GCNConv x2, eval mode) on 8 Trainium2 NeuronCores.

out = S @ (relu(S @ (x@W1) + b1) @ W2) + b2,  S = D^-1/2 (A+I) D^-1/2

Destination-sharded design (v3):
  - 50000 nodes sharded 6250/core; per core, destinations are permuted into
    49 blocks of 128 balanced by in-degree (host permutation, undone on the
    way out), so the SPMD program's per-block tile counts carry ~0.2%
    padding instead of ~13%.
  - h' = dinv*(x@W1) is computed sharded in bf16 (x arrives host-transposed
    so phase A is matmul+activation only) and AllGathered into a 256B-row
    table in Shared DRAM split into two chunks. The two layers use
    DIFFERENT chunk splits: layer 1's table has a SMALL chunk0 (20 blocks,
    fires ~40% into phase A, so L1 gathers start early) while layer 2's has
    a SMALL chunk1 (17 blocks, minimizing the serial AllGather tail after
    layer 1). A tiny warmup AllReduce at program start absorbs cross-core
    arrival skew before the first real collective.
  - Aggregation per 128-dest block: dma_gather source rows (<=1024 idx per
    call — the Q7 ucode dies above that, verified — round-robined over 4
    SWDGE queues), one-hot selectors built 1 op/block via a broadcast-AP
    is_equal (all-bf16), then matmul(lhsT=onehot[e,d], rhs=msgs[e,f])
    accumulating in PSUM. Gather calls for the first table chunk are
    emitted LAG groups ahead of second-chunk calls so a wait on the second
    AllGather can't head-of-line-block the in-order gpsimd queue.
    The reference's added self-loops never touch the gather path: an
    identity-selector matmul adds the block's own rows from a persistent
    SBUF copy (h1own/h2own), so they are never re-read from DRAM.
  - Layer 1 tail: h2pre = dinv*relu(dinv*G) = relu(dinv^2*G) in one
    ScalarE op (b1==0); AllGather -> layer-2 table of h2pre rows; layer 2
    aggregates h2pre and applies W2 after aggregation:
    out = dinv*(G2)@W2 + b2, accumulated feature-major in SBUF, stored in
    two bulk DMAs, and transposed on host.
  - int16 gather indices force a lo/hi base split between the two chunks.
"""

import math
from contextlib import ExitStack

import numpy as np

NC = 8
P = 128
GROUP = 4  # dest blocks per gather buffer
MAXCALL = 8  # tiles per dma_gather call (1024 idx ucode limit)
NQUEUES = 4
PAD_DEST = 200  # destid for padding edges; never matches iota 0..127
DEBUG_STAGE = "full"  # "A" | "AG1" | "L1" | "AG2" | "full"


def _pack_idx(v: np.ndarray) -> np.ndarray:
    """[T*128] int -> [128, 8T] int16 in dma_gather's wrap-16 layout,
    replicated over the 8 gpsimd cores (element i lives at [i%16, i//16])."""
    assert v.size % P == 0
    a = v.reshape(-1, 16).T.astype(np.int16)  # [16, 8T]
    return np.tile(a, (8, 1))  # [128, 8T]


def _balance_blocks(weights: np.ndarray, nblk: int) -> np.ndarray:
    """Assign len(weights) items into nblk blocks of <=128, balancing block
    weight sums. Returns pos[i] = block*128 + slot."""
    import heapq

    n = weights.size
    order = np.argsort(-weights, kind="stable")
    loads = np.zeros(nblk, dtype=np.int64)
    fill = np.zeros(nblk, dtype=np.int64)
    cap = np.full(nblk, P, dtype=np.int64)
    cap[nblk - 1] = n - (nblk - 1) * P  # last block holds the remainder
    pos = np.empty(n, dtype=np.int64)
    heap = [(0, b) for b in range(nblk)]
    heapq.heapify(heap)
    for i in order:
        while True:
            load, b = heapq.heappop(heap)
            if fill[b] < cap[b]:
                break
        pos[i] = b * P + fill[b]
        fill[b] += 1
        loads[b] = load + weights[i]
        if fill[b] < cap[b]:
            heapq.heappush(heap, (int(loads[b]), b))
    return pos


def _preprocess(x, edge_index, W1, b1, W2, b2):
    import ml_dtypes

    N, Fin = x.shape
    Fh = W1.shape[1]
    Fout = W2.shape[1]
    assert N % NC == 0
    NPC = N // NC
    NBLK = math.ceil(NPC / P)

    row = np.asarray(edge_index[0], dtype=np.int64)
    col = np.asarray(edge_index[1], dtype=np.int64)

    # degrees include the self-loop the reference appends to every node
    deg = (np.bincount(col, minlength=N) + 1).astype(np.float64)
    dinv = (1.0 / np.sqrt(deg)).astype(np.float32)

    core = col // NPC

    # per-core balanced permutation of destination slots by in-edge count
    cnt_in = np.bincount(col, minlength=N)
    pos_in_core = np.empty(N, dtype=np.int64)
    node_at = np.empty((NC, NPC), dtype=np.int64)
    for c in range(NC):
        w = cnt_in[c * NPC : (c + 1) * NPC]
        p = _balance_blocks(w, NBLK)
        pos_in_core[c * NPC : (c + 1) * NPC] = p
        node_at[c][p] = np.arange(NPC) + c * NPC

    # chunk-interleaved global row of every node in the gather tables:
    # AllGather chunk k covers core-local rows [r0,r1); its output holds the
    # 8 cores' slices consecutively at table rows [8*r0, 8*r1).
    # The two layers use DIFFERENT chunk splits: layer 1 wants a small chunk0
    # (it gates the L1 gather start right after phase A), layer 2 wants a
    # small chunk1 (it is the serial AllGather tail after layer 1).
    groups0 = [list(range(g, min(g + GROUP, NBLK))) for g in range(0, NBLK, GROUP)]
    blk = pos_in_core[col] >> 7
    dloc = pos_in_core[col] & 127

    def build_layout(cg):
        """Bucketing for one layer's table split [cg, rest] (in groups)."""
        chunks = []
        chunk_last_group = []
        for g0, g1 in ((0, cg), (cg, len(groups0))):
            blocks = [b for gr in groups0[g0:g1] for b in gr]
            r0 = blocks[0] * P
            r1 = min(blocks[-1] * P + P, NPC)
            chunks.append((r0, r1 - r0))
            chunk_last_group.append(g1 - 1)
        chunk_of_row = np.empty(NPC, dtype=np.int64)
        for k, (r0, ln) in enumerate(chunks):
            chunk_of_row[r0 : r0 + ln] = k
        k_of = chunk_of_row[pos_in_core]
        r0_of = np.array([c[0] for c in chunks])[k_of]
        len_of = np.array([c[1] for c in chunks])[k_of]
        gpos_node = (
            NC * r0_of + (np.arange(N) // NPC) * len_of + (pos_in_core - r0_of)
        )
        for r0k, lnk in chunks:
            assert NC * lnk <= 32768, "table half exceeds int16 index range"

        seg = k_of[row]  # which table half the source row lives in
        src_idx = gpos_node[row] - NC * r0_of[row]
        key = (core * NBLK + blk) * 2 + seg
        order = np.argsort(key, kind="stable")
        skey = key[order]
        nbuck = NC * NBLK * 2
        starts = np.searchsorted(skey, np.arange(nbuck))
        ends = np.searchsorted(skey, np.arange(nbuck) + 1)
        cnt = (ends - starts).reshape(NC, NBLK, 2)
        T_LO = np.maximum(1, np.ceil(cnt[:, :, 0] / P).max(axis=0).astype(np.int64))
        T_HI = np.maximum(1, np.ceil(cnt[:, :, 1] / P).max(axis=0).astype(np.int64))
        return dict(
            chunks=chunks,
            chunk_last_group=chunk_last_group,
            ssrc=src_idx[order],
            sdloc=dloc[order],
            starts=starts,
            ends=ends,
            T_LO=T_LO,
            T_HI=T_HI,
            T_consume=int((T_LO + T_HI).sum()),
        )

    ng = len(groups0)
    cg2 = min(math.ceil(ng / 2) + 1, ng - 1)
    while cg2 * GROUP * P * NC > 32768:
        cg2 -= 1
    cg1 = ng - cg2  # small chunk0 for layer 1 (mirror of layer 2's split)
    while (ng - cg1) * GROUP * P * NC > 32768:
        cg1 += 1
    lays = [build_layout(cg1), build_layout(cg2)]

    groups = groups0

    in_maps = []
    w1bf = np.asarray(W1, dtype=ml_dtypes.bfloat16)
    w2bf = np.asarray(W2, dtype=ml_dtypes.bfloat16)
    b1f = np.asarray(b1, dtype=np.float32)
    b2f = np.asarray(b2, dtype=np.float32)
    b1b = np.broadcast_to(b1f[None, :], (P, Fh)).copy()
    b2c = np.ascontiguousarray(b2f[:, None])  # [Fout, 1]

    for c in range(NC):
        did_parts = []
        idx_cols = []
        for lay in lays:
            TSEG = (lay["T_LO"], lay["T_HI"])
            starts, ends = lay["starts"], lay["ends"]
            ssrc, sdloc = lay["ssrc"], lay["sdloc"]
            did = np.full((P, lay["T_consume"]), PAD_DEST, dtype=np.float32)
            ccol = 0
            for b in range(NBLK):
                for sg in (0, 1):
                    bidx = (c * NBLK + b) * 2 + sg
                    n = ends[bidx] - starts[bidx]
                    T = int(TSEG[sg][b])
                    if T == 0:
                        assert n == 0
                        continue
                    tmp = np.full(T * P, PAD_DEST, dtype=np.float32)
                    tmp[:n] = sdloc[starts[bidx] : ends[bidx]]
                    did[:, ccol : ccol + T] = tmp.reshape(T, P).T
                    ccol += T
            assert ccol == lay["T_consume"]
            did_parts.append(did)

            for blocks in groups:
                for sg in (0, 1):
                    for b in blocks:
                        T = int(TSEG[sg][b])
                        if T == 0:
                            continue
                        bidx = (c * NBLK + b) * 2 + sg
                        n = ends[bidx] - starts[bidx]
                        s = ssrc[starts[bidx] : ends[bidx]]
                        tmp = np.zeros(T * P, dtype=np.int64)
                        tmp[:n] = s
                        idx_cols.append(_pack_idx(tmp))
        did = np.concatenate(did_parts, axis=1)
        idx = np.concatenate(idx_cols, axis=1)

        # dinv and dinv^2 columns at permuted positions (pad 1.0)
        dvflat = np.ones(NBLK * P, dtype=np.float32)
        dvflat[pos_in_core[c * NPC : (c + 1) * NPC]] = dinv[c * NPC : (c + 1) * NPC]
        dvc = np.ascontiguousarray(dvflat.reshape(NBLK, P).T)
        dv2c = np.ascontiguousarray(dvc * dvc)

        xs = np.zeros((NBLK * P, Fin), dtype=ml_dtypes.bfloat16)
        xs[:NPC] = x[node_at[c]].astype(ml_dtypes.bfloat16)
        # host-transposed: [Fin, NBLK*P]; block b's lhsT = [:, b*P:(b+1)*P],
        # so phase A needs no PE transpose / PSUM round-trip
        xsw = np.ascontiguousarray(xs.T)

        in_maps.append(
            {
                "xs": xsw,
                "w1": w1bf,
                "w2": w2bf,
                "b1b": b1b,
                "b2c": b2c,
                "dv": dvc,
                "dv2": dv2c,
                "idx": np.ascontiguousarray(idx),
                "did": did.astype(ml_dtypes.bfloat16),
            }
        )

    meta = dict(
        N=N,
        Fin=Fin,
        Fh=Fh,
        Fout=Fout,
        NPC=NPC,
        NBLK=NBLK,
        T_LO=[[int(t) for t in lay["T_LO"]] for lay in lays],
        T_HI=[[int(t) for t in lay["T_HI"]] for lay in lays],
        T_MAX=int(
            max(
                int(lay["T_LO"][b]) + int(lay["T_HI"][b])
                for lay in lays
                for b in range(NBLK)
            )
        ),
        groups=groups,
        chunks=[lay["chunks"] for lay in lays],
        chunk_last_group=[lay["chunk_last_group"] for lay in lays],
        idx_w=in_maps[0]["idx"].shape[1],
        T_consume=[lay["T_consume"] for lay in lays],
        has_b1=bool(np.any(b1f)),
    )
    return in_maps, meta, node_at


def _build_program(meta):
    import concourse.bacc as bacc
    import concourse.mybir as mybir
    import concourse.tile as tile
    from concourse.masks import make_identity

    N = meta["N"]
    Fin, Fh, Fout = meta["Fin"], meta["Fh"], meta["Fout"]
    NPC, NBLK = meta["NPC"], meta["NBLK"]
    groups = meta["groups"]
    TW = Fh  # table row width: 256B bf16 rows

    f32 = mybir.dt.float32
    bf16 = mybir.dt.bfloat16
    i16 = mybir.dt.int16
    i32 = mybir.dt.int32

    nc = bacc.Bacc(
        "TRN2", target_bir_lowering=False, debug=True, num_swdge_queues=NQUEUES
    )

    xs = nc.dram_tensor("xs", [Fin, NBLK * P], bf16, kind="ExternalInput")
    w1 = nc.dram_tensor("w1", [Fin, Fh], bf16, kind="ExternalInput")
    w2 = nc.dram_tensor("w2", [Fh, Fout], bf16, kind="ExternalInput")
    b1b = nc.dram_tensor("b1b", [P, Fh], f32, kind="ExternalInput")
    b2c = nc.dram_tensor("b2c", [Fout, 1], f32, kind="ExternalInput")
    dv = nc.dram_tensor("dv", [P, NBLK], f32, kind="ExternalInput")
    dv2 = nc.dram_tensor("dv2", [P, NBLK], f32, kind="ExternalInput")
    idxd = nc.dram_tensor("idx", [P, meta["idx_w"]], i16, kind="ExternalInput")
    did_w = sum(meta["T_consume"])
    didd = nc.dram_tensor("did", [P, did_w], bf16, kind="ExternalInput")
    out = nc.dram_tensor("out", [Fout, NPC], f32, kind="ExternalOutput")

    qctr = [0]

    with tile.TileContext(nc, num_cores=NC) as tc, ExitStack() as ctx:
        consts = ctx.enter_context(tc.tile_pool(name="consts", bufs=1))
        dram = ctx.enter_context(tc.tile_pool(name="dram", bufs=1, space="DRAM"))
        wpool = ctx.enter_context(tc.tile_pool(name="work", bufs=4))
        hpool = ctx.enter_context(tc.tile_pool(name="hp", bufs=4))
        ownp = ctx.enter_context(tc.tile_pool(name="hown", bufs=1))
        selp = ctx.enter_context(tc.tile_pool(name="sel", bufs=3))
        glo = ctx.enter_context(tc.tile_pool(name="glo", bufs=5))
        ghi = ctx.enter_context(tc.tile_pool(name="ghi", bufs=2))
        part = ctx.enter_context(tc.tile_pool(name="part", bufs=24))
        pg = ctx.enter_context(tc.tile_pool(name="pg", bufs=3, space="PSUM"))
        pt = ctx.enter_context(tc.tile_pool(name="pt", bufs=3, space="PSUM"))
        ph = ctx.enter_context(tc.tile_pool(name="ph", bufs=2, space="PSUM"))

        # ---- constants
        ident = consts.tile([P, P], f32)
        make_identity(nc, ident[:])
        ident_bf = consts.tile([P, P], bf16)
        nc.vector.tensor_copy(ident_bf[:], ident[:])
        iota_i = consts.tile([P, meta["T_MAX"], P], i32)
        nc.gpsimd.iota(
            iota_i[:], pattern=[[0, meta["T_MAX"]], [1, P]], base=0,
            channel_multiplier=0,
        )
        iota_f = consts.tile([P, meta["T_MAX"], P], bf16)
        nc.vector.tensor_copy(iota_f[:], iota_i[:])
        w1t = consts.tile([Fin, Fh], bf16)
        nc.sync.dma_start(w1t[:], w1[:])
        w2t = consts.tile([Fh, Fout], bf16)
        nc.sync.dma_start(w2t[:], w2[:])
        dvt = consts.tile([P, NBLK], f32)
        nc.sync.dma_start(dvt[:], dv[:])
        dv2t = consts.tile([P, NBLK], f32)
        nc.sync.dma_start(dv2t[:], dv2[:])
        didt = consts.tile([P, did_w], bf16)
        nc.sync.dma_start(didt[:], didd[:])
        idxt = consts.tile([P, meta["idx_w"]], i16)
        nc.sync.dma_start(idxt[:], idxd[:])
        b1t = consts.tile([P, Fh], f32)
        nc.sync.dma_start(b1t[:], b1b[:])
        b2t = consts.tile([Fout, 1], f32)
        nc.sync.dma_start(b2t[:], b2c[:])
        xst = consts.tile([Fin, NBLK * P], bf16)
        nc.sync.dma_start(xst[:], xs[:])
        outsb = consts.tile([Fout, NPC], f32)

        h1s = dram.tile([NPC, TW], bf16)
        h2s = dram.tile([NPC, TW], bf16)
        # warmup collective: absorbs cross-core arrival skew + ncfw startup
        # while constants load, so AG1 chunk 0 starts without a sync delay
        wu_in = dram.tile([1, P], bf16, name="wu_in")
        wu_out = dram.tile([1, P], bf16, addr_space="Shared", name="wu_out")
        nc.gpsimd.collective_compute(
            "AllReduce",
            mybir.AluOpType.add,
            replica_groups=[list(range(NC))],
            ins=[wu_in[:]],
            outs=[wu_out[:]],
        )
        # persistent SBUF copies of this core's own h rows (self-loop rhs);
        # avoids per-block 256B-descriptor DMA reloads from h1s/h2s
        h1own = ownp.tile([P, NBLK * Fh], bf16)
        h2own = ownp.tile([P, NBLK * Fh], bf16)
        t1h = [
            dram.tile([NC * ln, TW], bf16, addr_space="Shared", name=f"t1h{k}")
            for k, (_, ln) in enumerate(meta["chunks"][0])
        ]
        t2h = [
            dram.tile([NC * ln, TW], bf16, addr_space="Shared", name=f"t2h{k}")
            for k, (_, ln) in enumerate(meta["chunks"][1])
        ]

        def ag_chunk(shard, halves, chunks, k):
            r0, ln = chunks[k]
            nc.gpsimd.collective_compute(
                "AllGather",
                mybir.AluOpType.bypass,
                replica_groups=[list(range(NC))],
                ins=[shard[r0 : r0 + ln, :]],
                outs=[halves[k].opt()],
            )

        # ---- phase A: h' = dinv * (x @ W1), bf16, 256B rows;
        # AllGather chunk k fires as soon as its group's blocks are stored.
        for k, blocks in enumerate(groups):
            for b in blocks:
                nv = min(P, NPC - b * P)
                hps = pg.tile([P, Fh], f32, tag="pg")
                nc.tensor.matmul(
                    hps[:], lhsT=xst[:, b * P : (b + 1) * P], rhs=w1t[:],
                    start=True, stop=True,
                )
                hp = hpool.tile([P, TW], bf16, tag="hp")
                nc.scalar.activation(
                    hp[:, 0:Fh], hps[:], mybir.ActivationFunctionType.Copy,
                    scale=dvt[:, b : b + 1],
                )
                nc.vector.tensor_scalar(
                    h1own[:, b * Fh : (b + 1) * Fh], hps[:],
                    dvt[:, b : b + 1], None, mybir.AluOpType.mult,
                )
                nc.sync.dma_start(h1s[b * P : b * P + nv, :], hp[:nv, :])
            if DEBUG_STAGE in ("AG1", "L1", "AG2", "full"):
                if k in meta["chunk_last_group"][0]:
                    ag_chunk(
                        h1s, t1h, meta["chunks"][0],
                        meta["chunk_last_group"][0].index(k),
                    )

        # per-layer tile/idx/did offsets: layer 2's tables follow layer 1's
        l2_tiles = sum(
            meta["T_LO"][0][b] + meta["T_HI"][0][b] for b in range(NBLK)
        )

        def run_layer(layer):
            li = layer - 1
            T_LO, T_HI = meta["T_LO"][li], meta["T_HI"][li]
            halves = t1h if layer == 1 else t2h
            lo_ap = halves[0][:]
            hi_ap = halves[1][:] if len(halves) > 1 else None
            idx_off = 0 if layer == 1 else l2_tiles  # tiles, gather order

            def chunked_gather(buf, src_ap, ntiles, idx_off):
                c0 = 0
                while c0 < ntiles:
                    cn = min(MAXCALL, ntiles - c0)
                    nc.gpsimd.dma_gather(
                        buf[:, c0 : c0 + cn, :], src_ap,
                        idxt[:, 8 * (idx_off + c0) : 8 * (idx_off + c0 + cn)],
                        P * cn, P * cn, TW,
                        queue_num=qctr[0] % NQUEUES,
                    )
                    qctr[0] += 1
                    c0 += cn

            # software-pipeline gather emission: issue lo gathers LAG groups
            # ahead of hi gathers, so a hi call's wait on the table's second
            # AllGather chunk can't head-of-line-block the runnable lo stream
            # on the in-order gpsimd queue.
            offs = []
            o = idx_off
            for blocks in groups:
                tlo_g = sum(T_LO[b] for b in blocks)
                thi_g = sum(T_HI[b] for b in blocks)
                offs.append((o, o + tlo_g, tlo_g, thi_g))
                o += tlo_g + thi_g
            idx_off = o
            lobs = [None] * len(groups)

            def emit_lo(gk):
                lo_off, _, tlo_g, _ = offs[gk]
                lob = glo.tile([P, tlo_g, TW], bf16, tag="glo")
                chunked_gather(lob, lo_ap, tlo_g, lo_off)
                lobs[gk] = lob

            def emit_group(gk, blocks, split):
                lo_off, hi_off, tlo_g, thi_g = offs[gk]
                lob = lobs[gk]
                hib = None
                if thi_g > 0:
                    hib = ghi.tile([P, thi_g, TW], bf16, tag="ghi")
                    chunked_gather(hib, hi_ap, thi_g, hi_off)
                consume(gk, blocks, lob, hib, split)

            # per-block selector-plane offsets into didt (lo planes first,
            # then hi planes, per block)
            bdcol = []
            o2 = 0 if layer == 1 else meta["T_consume"][0]
            for b in range(NBLK):
                bdcol.append(o2)
                o2 += T_LO[b] + T_HI[b]

            def build_sel(c0, n):
                sel = selp.tile([P, n, P], bf16, tag="sel", name=f"sel_{c0}")
                nc.vector.tensor_tensor(
                    sel[:],
                    didt[:, c0 : c0 + n].to_broadcast([P, n, P]),
                    iota_f[:, 0:n, :],
                    mybir.AluOpType.is_equal,
                )
                return sel

            partials = {}

            def consume_lo(gk, blocks):
                """Split path: aggregate the lo half into a bf16 partial as
                soon as lob lands, releasing the lob buffer early and keeping
                the PE busy while the second AllGather chunk is in flight."""
                lob = lobs[gk]
                lo_t = 0
                for b in blocks:
                    ntl = T_LO[b]
                    sel = build_sel(bdcol[b], ntl)
                    own = (h1own if layer == 1 else h2own)[
                        :, b * Fh : (b + 1) * Fh
                    ]
                    acc = pg.tile([P, Fh], f32, tag="pg")
                    nc.tensor.matmul(
                        acc[:], lhsT=ident_bf[:], rhs=own,
                        start=True, stop=False,
                    )
                    for t in range(ntl):
                        nc.tensor.matmul(
                            acc[:], lhsT=sel[:, t, :],
                            rhs=lob[:, lo_t + t, 0:Fh],
                            start=False, stop=(t == ntl - 1),
                        )
                    pb = part.tile([P, Fh], bf16, tag="part", name=f"pb{b}")
                    nc.vector.tensor_copy(pb[:], acc[:])
                    partials[b] = pb
                    lo_t += ntl

            def consume(gk, blocks, lob, hib, split):
                lo_t = 0
                hi_t = 0
                for b in blocks:
                    nv = min(P, NPC - b * P)
                    ntile = T_LO[b] + T_HI[b]

                    acc = pg.tile([P, Fh], f32, tag="pg")
                    if split:
                        # resume from the parked lo-partial
                        sel = build_sel(bdcol[b] + T_LO[b], T_HI[b])
                        nc.tensor.matmul(
                            acc[:], lhsT=ident_bf[:], rhs=partials[b][:],
                            start=True, stop=False,
                        )
                        k = 0
                        tiles = ((hib, hi_t, T_HI[b]),)
                        klast = T_HI[b] - 1
                    else:
                        sel = build_sel(bdcol[b], ntile)
                        own = (h1own if layer == 1 else h2own)[
                            :, b * Fh : (b + 1) * Fh
                        ]
                        nc.tensor.matmul(
                            acc[:], lhsT=ident_bf[:], rhs=own,
                            start=True, stop=False,
                        )
                        k = 0
                        tiles = ((lob, lo_t, T_LO[b]), (hib, hi_t, T_HI[b]))
                        klast = ntile - 1
                    for buf, t0, tn in tiles:
                        for t in range(tn):
                            nc.tensor.matmul(
                                acc[:],
                                lhsT=sel[:, k, :],
                                rhs=buf[:, t0 + t, 0:Fh],
                                start=False,
                                stop=(k == klast),
                            )
                            k += 1
                    lo_t += T_LO[b]
                    hi_t += T_HI[b]

                    if layer == 1:
                        # h2pre = dinv*relu(dinv*G + b1) = relu(dinv^2*G), b1=0
                        hp = hpool.tile([P, TW], bf16, tag="hp")
                        if meta["has_b1"]:
                            tmp = wpool.tile([P, Fh], f32, tag="l1tmp")
                            nc.vector.tensor_scalar(
                                tmp[:], acc[:], dvt[:, b : b + 1], None,
                                mybir.AluOpType.mult,
                            )
                            nc.vector.tensor_tensor(
                                tmp[:], tmp[:], b1t[:], mybir.AluOpType.add
                            )
                            nc.scalar.activation(
                                hp[:, 0:Fh], tmp[:],
                                mybir.ActivationFunctionType.Relu,
                                scale=dvt[:, b : b + 1],
                            )
                        else:
                            nc.scalar.activation(
                                hp[:, 0:Fh], acc[:],
                                mybir.ActivationFunctionType.Relu,
                                scale=dv2t[:, b : b + 1],
                            )
                        if meta["has_b1"]:
                            nc.vector.tensor_copy(
                                h2own[:, b * Fh : (b + 1) * Fh], hp[:, 0:Fh]
                            )
                        else:
                            nc.scalar.activation(
                                h2own[:, b * Fh : (b + 1) * Fh], acc[:],
                                mybir.ActivationFunctionType.Relu,
                                scale=dv2t[:, b : b + 1],
                            )
                        nc.sync.dma_start(
                            h2s[b * P : b * P + nv, :], hp[:nv, :]
                        )
                    else:
                        # out.T[:, block] = W2.T @ (dinv*G2).T + b2
                        r2 = wpool.tile([P, Fh], f32, tag="l2r")
                        nc.scalar.activation(
                            r2[:], acc[:], mybir.ActivationFunctionType.Copy,
                            scale=dvt[:, b : b + 1],
                        )
                        r2T_ps = pt.tile([P, P], f32, tag="pt")
                        nc.tensor.transpose(r2T_ps[:], r2[:], ident[:])
                        r2T = wpool.tile([P, P], bf16, tag="wbf")
                        nc.vector.tensor_copy(r2T[:], r2T_ps[:])
                        o2T_ps = ph.tile([Fout, P], f32, tag="ph")
                        nc.tensor.matmul(
                            o2T_ps[:], lhsT=w2t[:], rhs=r2T[:],
                            start=True, stop=True,
                        )
                        nc.scalar.activation(
                            outsb[:, b * P : b * P + nv], o2T_ps[:, :nv],
                            mybir.ActivationFunctionType.Identity,
                            bias=b2t[:, 0:1],
                        )
                        if b == 31:
                            nc.sync.dma_start(
                                out[:, : 32 * P], outsb[:, : 32 * P]
                            )
                if layer == 1 and DEBUG_STAGE in ("AG2", "full"):
                    if gk in meta["chunk_last_group"][1]:
                        ag_chunk(
                            h2s, t2h, meta["chunks"][1],
                            meta["chunk_last_group"][1].index(gk),
                        )

            LAG = 3
            SPLIT = 5  # groups using split lo/hi consumption
            ng = len(groups)
            for gk in range(min(LAG, ng)):
                emit_lo(gk)
                if gk < SPLIT:
                    consume_lo(gk, groups[gk])
            for gk, blocks in enumerate(groups):
                if gk + LAG < ng:
                    emit_lo(gk + LAG)
                    if gk + LAG < SPLIT:
                        consume_lo(gk + LAG, groups[gk + LAG])
                emit_group(gk, blocks, gk < SPLIT)

        if DEBUG_STAGE in ("L1", "AG2", "full"):
            run_layer(1)
        if DEBUG_STAGE == "full":
            run_layer(2)
            nc.sync.dma_start(out[:, 32 * P :], outsb[:, 32 * P :])

        if DEBUG_STAGE != "full":
            # debug: write finite bytes into `out` derived from stage product
            src = {"A": h1s, "AG1": t1h[0], "L1": h2s, "AG2": t2h[0]}[DEBUG_STAGE]
            gdb = wpool.tile([P, Fout], bf16, tag="wbf")
            nc.sync.dma_start(gdb[:], src[0:P, 0:Fout])
            red = wpool.tile([P, 1], f32, tag="l1tmp")
            nc.vector.reduce_sum(red[:], gdb[:], axis=mybir.AxisListType.X)
            odb = wpool.tile([Fout, NPC], f32, tag="l2o")
            nc.vector.memset(odb[:], 0.0)
            nc.vector.tensor_copy(odb[0:1, 0:1], red[0:1, :])
            nc.sync.dma_start(out[:], odb[:])

    nc.compile()
    return nc


def _assemble(results, meta, node_at):
    N, Fout = meta["N"], meta["Fout"]
    out = np.empty((N, Fout), dtype=np.float32)
    for c in range(NC):
        out[node_at[c]] = np.asarray(results[c]["out"]).T
    return out


def kernel(**inputs) -> np.ndarray:
    x = np.asarray(inputs["x"])
    edge_index = np.asarray(inputs["edge_index"])
    W1 = np.asarray(inputs["W1"])
    b1 = np.asarray(inputs["b1"])
    W2 = np.asarray(inputs["W2"])
    b2 = np.asarray(inputs["b2"])

    in_maps, meta, node_at = _preprocess(x, edge_index, W1, b1, W2, b2)
    nc = _build_program(meta)

    from concourse.bass_utils import run_bass_kernel_spmd

    res = run_bass_kernel_spmd(nc, in_maps, list(range(NC)))
    return _assemble(res.results, meta, node_at)

